# revision 14
# baseline (speedup 1.0000x reference)
"""MoE CouncilLayer kernel for 8x TRN2 NeuronCores (expert-parallel, fp8).

Problem (all-expert MoE, B=2, T=1024, C=768, E=32, H=3072):
    gates = softmax(x @ gate_w + gate_b)                     # [N, E]
    h     = gelu(einsum('nc,ech->neh', x, w1) + b1)          # [N, E, H]
    y     = einsum('neh,ehc->nec', h, w2) + b2               # [N, E, C]
    out   = einsum('ne,nec->nc', gates, y)                   # [N, C]

Sharding: expert-parallel, 4 experts per core; x replicated. Each core
computes its 4 experts' gate-weighted partial sum; host adds the 8
partials.

All matmuls run as fp8e4 (e4m3) DoubleRow matmuls: each instruction
contracts two K=128 blocks at 0.5 cycles per output column - 4x the
fp16 MAC rate. Accuracy is recovered with Dekker-style 2-term fp8
splits of every operand (hi = q8(a), lo = q8(a - hi)) and a 3-product
scheme per logical matmul (hi*hi + lo*hi + hi*lo; the lo*lo term is
dropped), all accumulating in one fp32 PSUM group:
    mm1: h_psum = (x_hi + x_lo) @ w1_hi + x_hi @ w1_lo      (18 K128
         products per [128h x 512t] tile = 9 DoubleRow matmuls)
    mm2: y_psum = (hg_hi + hg_lo) @ w2_hi + hg_hi @ w2_lo
Weights are pre-scaled on the host before quantization (w1 x64,
w2 x128) so their sigma sits mid-range in e4m3 instead of at the
subnormal floor; the inverse scales fold into the ACT gelu input scale
and the gate constants (softmax 'ones' weights = 128 so the on-device
gates come out as g/128, cancelling mm2's x128 psum scale). Measured
end-to-end rel RMS error of this scheme is ~2e-3 (budget 2e-2); PE
time is 6/8 of the fp16 stream = ~740us vs the 983us fp16 floor.

Per-core dataflow per (token-group, expert):
    mm1 DR stream -> psum; ACT pass1 gelu->fp16 scratch, ACT pass2
    gelu->fp8 (hg_hi); DVE subtract -> hg_lo fp8. mm2 DR stream ->
    psum; Pool engine folds (psum + 128*b2) * g_bcast into the fp32
    yac accumulator (scalar_tensor_tensor + tensor_add), keeping DVE
    free for the lo-extraction.

Gates are computed on-device: logits via fp8 DoubleRow Dekker matmuls
(gw_hi/gw_lo x64 host-scaled, 3-product scheme -> ~0.1% logit error),
col-tiled per 512-token chunk in borrowed mm2 psum banks; fp32 ACT exp
(input scale 1/64, bias gate_b), fp16 ones-matmul denominators, DVE
reciprocal; [128, N] per-expert gate broadcast via a DRAM bounce.
Gate columns are permuted host-side so each core's 4 local experts sit
at columns 0..3 (keeps the SPMD program core-agnostic).

Head scheduling: the first four h-blocks of mm1 are traced
stream-outer/cc-outer across all 8 psum banks so PE consumes each
arriving x_hi/x_lo/w1_lo chunk as it lands; the gate softmax is traced
behind them and drains on ACT/DVE under the matmul stream.
"""

import numpy as np
import ml_dtypes

import concourse.tile as tile
from concourse import bacc, mybir
from concourse.bass_utils import run_bass_kernel_spmd

# Problem dims (hardcoded per harness contract)
B, T, C, E, H = 2, 1024, 768, 32, 3072
N = B * T  # 2048 tokens
NCORES = 8
EL = E // NCORES  # 4 local experts
CB = C // 128  # 6 c-blocks
HB = H // 128  # 24 h-blocks
CP = CB // 2  # 3 cc-pairs (DoubleRow K pairs)
HP = HB // 2  # 12 hb-pairs
TCG = 2  # token groups (1024 each)
TG = N // TCG  # 1024
TI = TG // 512  # 512-token chunks per group

S1 = 64.0  # host pre-scale on w1 (and gate_w) before fp8 quantization
S2 = 128.0  # host pre-scale on w2; also folded into softmax denominators

F8 = mybir.dt.float8e4
F16 = mybir.dt.float16
F32 = mybir.dt.float32
AF = mybir.ActivationFunctionType
DR = mybir.MatmulPerfMode.DoubleRow
NP8 = ml_dtypes.float8_e4m3

_CACHED_NC = None


def build_nc(act=AF.Gelu):
    nc = bacc.Bacc(trn_type="TRN2")

    xh_d = nc.dram_tensor("xh", [TCG, 128, CB * TG], F8, kind="ExternalInput")
    xl_d = nc.dram_tensor("xl", [TCG, 128, CB * TG], F8, kind="ExternalInput")
    gwh_d = nc.dram_tensor("gwh", [C, E], F8, kind="ExternalInput")
    gwl_d = nc.dram_tensor("gwl", [C, E], F8, kind="ExternalInput")
    gb_d = nc.dram_tensor("gb", [E, 1], F32, kind="ExternalInput")
    ones_d = nc.dram_tensor("ones32", [E, EL], F16, kind="ExternalInput")
    # w1/w2 are host-packed so every per-(expert, tile) DMA reads one
    # contiguous 3072B run per partition (<512B runs pay a 2x DMA penalty)
    w1h_d = nc.dram_tensor("w1h", [EL, HB // 4, 128, CB * 512], F8, kind="ExternalInput")
    w1l_d = nc.dram_tensor("w1l", [EL, HB // 4, 128, CB * 512], F8, kind="ExternalInput")
    b1_d = nc.dram_tensor("b1", [128, EL, HB], F32, kind="ExternalInput")
    w2h_d = nc.dram_tensor("w2h", [EL, CB, 128, HB * 128], F8, kind="ExternalInput")
    w2l_d = nc.dram_tensor("w2l", [EL, CB, 128, HB * 128], F8, kind="ExternalInput")
    b2P_d = nc.dram_tensor("b2P", [128, EL, CB], F32, kind="ExternalInput")
    outT_d = nc.dram_tensor("outT", [C, N], F32, kind="ExternalOutput")

    def w1_ap(dram, e, hbg):
        return dram[e, hbg, :, :].rearrange("p (cc h) -> p cc h", h=512)

    def w2_ap(dram, e, cb):
        return dram[e, cb, :, :].rearrange("p (hb c) -> p hb c", c=128)

    def x_ap(dram, tg):
        return dram[tg, :, :].rearrange("p (cc t) -> p cc t", t=TG)

    with tile.TileContext(nc) as tc:
        with (
            tc.tile_pool(name="const", bufs=1) as cp,
            tc.tile_pool(name="stream", bufs=1) as sp,
            tc.tile_pool(name="psum", bufs=1, space="PSUM") as pp,
            tc.tile_pool(name="dram", bufs=1, space="DRAM") as dp,
        ):
            # --- resident tiles ---
            xh_sb = cp.tile([128, CB, N], F8)
            xl_sb = cp.tile([128, CB, N], F8)
            gwh_sb = cp.tile([128, CB, E], F8)
            gwl_sb = cp.tile([128, CB, E], F8)
            gb_sb = cp.tile([E, 1], F32)
            ones_sb = cp.tile([E, EL], F16)
            b1_sb = cp.tile([128, EL, HB], F32)
            b2P_sb = cp.tile([128, EL, CB], F32)
            expT_sb = cp.tile([E, N], F32)
            expT16_sb = cp.tile([E, N], F16)
            g_bcast_sb = cp.tile([128, EL, N], F16)
            g_localT_sb = cp.tile([EL, N], F16)

            # DMA issue order = arrival order. The specially-traced first
            # mm1 block consumes, in order: w1h tile + x_hi (stream A,
            # cc-outer), x_lo (stream B), w1l tile (stream C). b1 rides
            # after the first x_hi chunk (needed by the first gelu); gate
            # consts next (softmax is traced right behind block 0); then
            # the second w1h/w1l tiles and the x halves for token-group 1.
            w1h_first = sp.tile([128, CB, 512], F8, tag="w1", bufs=6, name="w1t")
            w1l_first = sp.tile([128, CB, 512], F8, tag="w1", bufs=6, name="w1t")
            ccs = [slice(0, 2), slice(2, 4), slice(4, 6)]
            for ci, cs in enumerate(ccs):
                nc.sync.dma_start(w1h_first[:, cs, :], w1_ap(w1h_d, 0, 0)[:, cs, :])
                nc.sync.dma_start(xh_sb[:, cs, 0:TG], x_ap(xh_d, 0)[:, cs, :])
                if ci == 0:
                    nc.sync.dma_start(b1_sb, b1_d[:, :, :])
            nc.sync.dma_start(xl_sb[:, :, 0:TG], x_ap(xl_d, 0)[:, :, :])
            nc.sync.dma_start(w1l_first, w1_ap(w1l_d, 0, 0))
            nc.sync.dma_start(gwh_sb, gwh_d[:, :].rearrange("(cc p) e -> p cc e", p=128))
            nc.sync.dma_start(gwl_sb, gwl_d[:, :].rearrange("(cc p) e -> p cc e", p=128))
            nc.sync.dma_start(gb_sb, gb_d[:, :])
            nc.sync.dma_start(ones_sb, ones_d[:, :])
            # prefetch e0's second w1 tile pair ahead of the bulk second-half
            # x transfer so mm1 hbg1 isn't gated on it
            w1h_second = sp.tile([128, CB, 512], F8, tag="w1", bufs=6, name="w1t")
            w1l_second = sp.tile([128, CB, 512], F8, tag="w1", bufs=6, name="w1t")
            nc.sync.dma_start(w1h_second, w1_ap(w1h_d, 0, 1))
            nc.sync.dma_start(w1l_second, w1_ap(w1l_d, 0, 1))
            # token-group 1 x halves and b2P aren't needed for ~60us+; keep
            # them off the weight-streaming queue (Pool engine's queue)
            nc.gpsimd.dma_start(xh_sb[:, :, TG:N], x_ap(xh_d, 1)[:, :, :])
            nc.gpsimd.dma_start(xl_sb[:, :, TG:N], x_ap(xl_d, 1)[:, :, :])
            nc.gpsimd.dma_start(b2P_sb, b2P_d[:, :, :])

            def emit_softmax():
                # logits via fp8 DR Dekker matmuls into borrowed tag-"y"
                # psum slots (mm2 doesn't need them until ~130us in). Each
                # 512-token chunk gets its own psum bank at partition
                # offset 32*t4 (col-tiled so chunk groups run concurrently
                # on PE sub-arrays).
                lgs = [
                    pp.tile([128, 512], F32, tag="y", bufs=4, name="lg")
                    for _ in range(4)
                ]
                for t4 in range(4):
                    ts = slice(t4 * 512, (t4 + 1) * 512)
                    out = lgs[t4][0:32, :]
                    n_st = 3 * CP
                    i = 0
                    for cpi in range(CP):
                        cs = slice(2 * cpi, 2 * cpi + 2)
                        for gw_t, x_t in (
                            (gwh_sb, xh_sb),
                            (gwh_sb, xl_sb),
                            (gwl_sb, xh_sb),
                        ):
                            nc.tensor.matmul(
                                out,
                                gw_t[:, cs, :],
                                x_t[:, cs, ts],
                                start=(i == 0),
                                stop=(i == n_st - 1),
                                perf_mode=DR,
                            )
                            i += 1
                for t4 in range(N // 512):
                    ts = slice(t4 * 512, (t4 + 1) * 512)
                    lgs4 = lgs[t4][0:32, :]
                    nc.scalar.activation(
                        expT_sb[:, ts], lgs4, AF.Exp, bias=gb_sb, scale=1.0 / S1
                    )
                    nc.scalar.activation(
                        expT16_sb[:, ts], lgs4, AF.Exp, bias=gb_sb, scale=1.0 / S1
                    )
                # denominators (x S2 via the ones constant): second pass so
                # the exps have drained on ACT by the time PE reaches these
                dns = [
                    pp.tile([128, 512], F32, tag="y", bufs=4, name="dn")
                    for _ in range(4)
                ]
                for t4 in range(N // 512):
                    ts = slice(t4 * 512, (t4 + 1) * 512)
                    nc.tensor.matmul(
                        dns[t4][32 * t4 : 32 * t4 + EL, :],
                        ones_sb[:, :],
                        expT16_sb[:, ts],
                        start=True,
                        stop=True,
                        tile_position=(0, 32 * t4),
                    )
                for t4 in range(N // 512):
                    ts = slice(t4 * 512, (t4 + 1) * 512)
                    rc = sp.tile([EL, 512], F32, tag="recip", bufs=2, name="rc")
                    nc.vector.reciprocal(rc, dns[t4][32 * t4 : 32 * t4 + EL, :])
                    nc.vector.tensor_mul(g_localT_sb[:, ts], expT_sb[0:EL, ts], rc)
                g_dram = dp.tile([EL, N], F16, name="g_dram")
                nc.gpsimd.dma_start(g_dram, g_localT_sb[:, :])
                for j in range(EL):
                    nc.gpsimd.dma_start(
                        g_bcast_sb[:, j, :],
                        g_dram[j : j + 1, :].to_broadcast((128, N)),
                    )

            def emit_gelu_split(e, hb, hps, hgh, hgl):
                # psum -> fp16 scratch + fp8 hi (ACT), fp8 lo (DVE)
                for ti in range(TI):
                    lts = slice(ti * 512, (ti + 1) * 512)
                    hf = sp.tile([128, 512], F16, tag="hf", bufs=4, name="hf")
                    bias = b1_sb[:, e, hb : hb + 1]
                    nc.scalar.activation(hf, hps[ti], act, bias=bias, scale=1.0 / S1)
                    nc.scalar.activation(
                        hgh[:, hb, lts], hps[ti], act, bias=bias, scale=1.0 / S1
                    )
                    nc.vector.tensor_tensor(
                        hgl[:, hb, lts], hf, hgh[:, hb, lts], mybir.AluOpType.subtract
                    )

            # mm1 stream/cc-pair schedule: the x_hi@w1_lo correction is
            # dropped on cc-pair 0 (K[0:256]) - residual 2.75%/sqrt(3) =
            # 1.6% rel RMS (mirror-verified 0.01605 end to end vs the 2e-2
            # gate) - saving 1 of 9 DR matmuls per mm1 tile (~41us).
            def mm1_pairs(w1h_t, w1l_t):
                out = []
                for si, (w_t, x_t) in enumerate(
                    ((w1h_t, xh_sb), (w1h_t, xl_sb), (w1l_t, xh_sb))
                ):
                    for cpi in range(CP):
                        if si == 2 and cpi == 0:
                            continue
                        out.append((si, cpi, w_t, x_t))
                return out

            def emit_mm1_first(hgh, hgl):
                # first 4 h-blocks of (tg0, e0), traced stream-outer and
                # cc-outer across all 8 psum banks: PE's in-order stream
                # consumes each arriving x_hi chunk, then x_lo, then w1l.
                hps8 = [
                    [
                        pp.tile(
                            [128, 512],
                            F32,
                            tag=("h" if hbi < 2 else "y"),
                            bufs=4,
                            name="hps",
                        )
                        for _ in range(TI)
                    ]
                    for hbi in range(4)
                ]
                pairs = mm1_pairs(w1h_first, w1l_first)
                for pi, (si, cpi, w_t, x_t) in enumerate(pairs):
                    cs = slice(2 * cpi, 2 * cpi + 2)
                    for ti in range(TI):
                        for hbi in range(4):
                            nc.tensor.matmul(
                                hps8[hbi][ti],
                                w_t[:, cs, hbi * 128 : (hbi + 1) * 128],
                                x_t[:, cs, ti * 512 : (ti + 1) * 512],
                                start=(pi == 0),
                                stop=(pi == len(pairs) - 1),
                                perf_mode=DR,
                            )
                for hbi in range(4):
                    emit_gelu_split(0, hbi, hps8[hbi], hgh, hgl)

            def emit_mm1(tg, e, hgh, hgl, hbg_start=0, hbg_end=HB // 4, pre=None):
                # mm1: h_psum = (x_hi + x_lo) @ w1_hi + x_hi @ w1_lo[K256:]
                for hbg in range(hbg_start, hbg_end):
                    if tg == 0 and e == 0 and hbg == 1:
                        w1h_t, w1l_t = w1h_second, w1l_second
                    elif hbg == hbg_start and pre is not None:
                        w1h_t, w1l_t = pre
                    else:
                        w1h_t = sp.tile([128, CB, 512], F8, tag="w1", bufs=6, name="w1t")
                        nc.sync.dma_start(w1h_t, w1_ap(w1h_d, e, hbg))
                        w1l_t = sp.tile([128, CB, 512], F8, tag="w1", bufs=6, name="w1t")
                        nc.sync.dma_start(w1l_t, w1_ap(w1l_d, e, hbg))
                    pairs = mm1_pairs(w1h_t, w1l_t)
                    for hbi in range(4):
                        hb = hbg * 4 + hbi
                        hps = [
                            pp.tile([128, 512], F32, tag="h", bufs=4, name="hps")
                            for _ in range(TI)
                        ]
                        for ti in range(TI):
                            gts = slice(tg * TG + ti * 512, tg * TG + (ti + 1) * 512)
                            for pi, (si, cpi, w_t, x_t) in enumerate(pairs):
                                cs = slice(2 * cpi, 2 * cpi + 2)
                                nc.tensor.matmul(
                                    hps[ti],
                                    w_t[:, cs, hbi * 128 : (hbi + 1) * 128],
                                    x_t[:, cs, gts],
                                    start=(pi == 0),
                                    stop=(pi == len(pairs) - 1),
                                    perf_mode=DR,
                                )
                        emit_gelu_split(e, hb, hps, hgh, hgl)

            def emit_mm2(tg, e, hgh, hgl, yac):
                # mm2: y_psum = (hg_hi + hg_lo) @ w2_hi + hg_hi @ w2_lo;
                # then yac (+)= (y_psum + S2*b2) * g_bcast: the psum-reading
                # stt on DVE (GPSIMD can't access PSUM), the SBUF-only
                # cross-expert add on the otherwise-idle Pool engine
                for cb in range(CB):
                    w2h_t = sp.tile([128, HB, 128], F8, tag="w2", bufs=8, name="w2t")
                    w2l_t = sp.tile([128, HB, 128], F8, tag="w2", bufs=8, name="w2t")
                    nc.sync.dma_start(w2h_t, w2_ap(w2h_d, e, cb))
                    nc.sync.dma_start(w2l_t, w2_ap(w2l_d, e, cb))
                    yps = [
                        pp.tile([128, 512], F32, tag="y", bufs=4, name="yps")
                        for _ in range(TI)
                    ]
                    for ti in range(TI):
                        lts = slice(ti * 512, (ti + 1) * 512)
                        i = 0
                        for hpi in range(HP):
                            hs = slice(2 * hpi, 2 * hpi + 2)
                            for w_t, h_t in (
                                (w2h_t, hgh),
                                (w2h_t, hgl),
                                (w2l_t, hgh),
                            ):
                                nc.tensor.matmul(
                                    yps[ti],
                                    w_t[:, hs, :],
                                    h_t[:, hs, lts],
                                    start=(i == 0),
                                    stop=(i == 3 * HP - 1),
                                    perf_mode=DR,
                                )
                                i += 1
                    for ti in range(TI):
                        gts = slice(tg * TG + ti * 512, tg * TG + (ti + 1) * 512)
                        lts = slice(ti * 512, (ti + 1) * 512)
                        if e == 0:
                            nc.vector.scalar_tensor_tensor(
                                out=yac[:, cb, lts],
                                in0=yps[ti],
                                scalar=b2P_sb[:, e, cb : cb + 1],
                                in1=g_bcast_sb[:, e, gts],
                                op0=mybir.AluOpType.add,
                                op1=mybir.AluOpType.mult,
                            )
                        else:
                            yt = sp.tile([128, 512], F32, tag="ytmp", bufs=2, name="yt")
                            nc.vector.scalar_tensor_tensor(
                                out=yt,
                                in0=yps[ti],
                                scalar=b2P_sb[:, e, cb : cb + 1],
                                in1=g_bcast_sb[:, e, gts],
                                op0=mybir.AluOpType.add,
                                op1=mybir.AluOpType.mult,
                            )
                            nc.gpsimd.tensor_tensor(
                                yac[:, cb, lts], yt, yac[:, cb, lts], mybir.AluOpType.add
                            )

            # --- main. Trace order = PE order: the special first block
            # (fills the x/w1 arrival window), the gate prologue (drains on
            # ACT/DVE under the matmul stream), then the expert stream.
            prefetched = {}
            for tg in range(TCG):
                hgh = sp.tile([128, HB, TG], F8, tag="hgh", bufs=1, name="hgh")
                hgl = sp.tile([128, HB, TG], F8, tag="hgl", bufs=1, name="hgl")
                yac = sp.tile([128, CB, TG], F32, tag="yacc", bufs=1, name="yac")
                for e in range(EL):
                    if tg == 0 and e == 0:
                        emit_mm1_first(hgh, hgl)
                        # hbg1 before the softmax: keeps PE fed while the
                        # gate exps/denominators serialize on ACT (gates are
                        # applied post-mm2, so they're not needed until ~60us)
                        emit_mm1(tg, e, hgh, hgl, hbg_start=1, hbg_end=2)
                        emit_softmax()
                        emit_mm1(tg, e, hgh, hgl, hbg_start=2)
                    else:
                        emit_mm1(tg, e, hgh, hgl, pre=prefetched.pop((tg, e), None))
                    # prefetch the next block's first w1 tile pair ahead of
                    # mm2's w2 DMA burst so the expert boundary doesn't stall
                    ntg, ne = (tg, e + 1) if e + 1 < EL else (tg + 1, 0)
                    if ntg < TCG:
                        w1h_p = sp.tile([128, CB, 512], F8, tag="w1", bufs=6, name="w1t")
                        nc.sync.dma_start(w1h_p, w1_ap(w1h_d, ne, 0))
                        w1l_p = sp.tile([128, CB, 512], F8, tag="w1", bufs=6, name="w1t")
                        nc.sync.dma_start(w1l_p, w1_ap(w1l_d, ne, 0))
                        prefetched[(ntg, ne)] = (w1h_p, w1l_p)
                    emit_mm2(tg, e, hgh, hgl, yac)
                for cb in range(CB):
                    for ti in range(TI):
                        nc.gpsimd.dma_start(
                            outT_d[
                                cb * 128 : (cb + 1) * 128,
                                tg * TG + ti * 512 : tg * TG + (ti + 1) * 512,
                            ],
                            yac[:, cb, ti * 512 : (ti + 1) * 512],
                        )

    nc.compile()
    return nc


def _get_nc():
    global _CACHED_NC
    if _CACHED_NC is None:
        _CACHED_NC = build_nc()
    return _CACHED_NC


def _split8(a, scale):
    """Dekker 2-term fp8 split of scale*a. Returns (hi, lo) as e4m3."""
    s = (a * scale).astype(np.float32)
    hi = s.astype(NP8)
    lo = (s - hi.astype(np.float32)).astype(NP8)
    return hi, lo


def make_in_maps(x, gate_w, gate_b, w1, b1, w2, b2):
    x = np.asarray(x, np.float32)
    gate_w = np.asarray(gate_w, np.float32)
    gate_b = np.asarray(gate_b, np.float32)
    w1 = np.asarray(w1, np.float32)
    b1 = np.asarray(b1, np.float32)
    w2 = np.asarray(w2, np.float32)
    b2 = np.asarray(b2, np.float32)

    # pack x as [tg, p, cc*TG]: contiguous 1KB+ runs per partition
    xT = x.reshape(N, C).T.reshape(CB, 128, TCG, TG).transpose(2, 1, 0, 3)
    xT = np.ascontiguousarray(xT).reshape(TCG, 128, CB * TG)
    xh, xl = _split8(xT, 1.0)

    ones32 = np.full((E, EL), S2, np.float16)

    in_maps = []
    for i in range(NCORES):
        lo, hi = EL * i, EL * (i + 1)
        perm = list(range(lo, hi)) + [e for e in range(E) if not (lo <= e < hi)]
        gwh, gwl = _split8(np.ascontiguousarray(gate_w[:, perm]), S1)
        # w1 packed [e, hbg, p, cc*512]; w2 packed [e, cb, p, hb*128]
        w1p = w1[lo:hi].reshape(EL, CB, 128, HB // 4, 512).transpose(0, 3, 2, 1, 4)
        w1p = np.ascontiguousarray(w1p).reshape(EL, HB // 4, 128, CB * 512)
        w2p = w2[lo:hi].reshape(EL, HB, 128, CB, 128).transpose(0, 3, 2, 1, 4)
        w2p = np.ascontiguousarray(w2p).reshape(EL, CB, 128, HB * 128)
        w1h, w1l = _split8(w1p, S1)
        w2h, w2l = _split8(w2p, S2)
        in_maps.append(
            {
                "xh": xh,
                "xl": xl,
                "gwh": gwh,
                "gwl": gwl,
                "gb": np.ascontiguousarray(gate_b[perm]).reshape(E, 1),
                "ones32": ones32,
                "w1h": w1h,
                "w1l": w1l,
                "b1": np.ascontiguousarray(
                    b1[lo:hi].reshape(EL, HB, 128).transpose(2, 0, 1)
                ),
                "w2h": w2h,
                "w2l": w2l,
                "b2P": np.ascontiguousarray(
                    (S2 * b2[lo:hi]).reshape(EL, CB, 128).transpose(2, 0, 1)
                ),
            }
        )
    return in_maps


def kernel(x, gate_w, gate_b, w1, b1, w2, b2, _trace=False, _tmpdir=None):
    nc = _get_nc()
    in_maps = make_in_maps(x, gate_w, gate_b, w1, b1, w2, b2)
    res = run_bass_kernel_spmd(
        nc,
        in_maps,
        core_ids=list(range(NCORES)),
        trace=_trace,
        tmpdir=_tmpdir,
    )
    acc = res.results[0]["outT"].astype(np.float64)
    for r in res.results[1:]:
        acc += r["outT"]
    out = acc.T.reshape(B, T, C).astype(np.float32)
    if _trace:
        kernel._last_results = res
    return out


# revision 15
# speedup vs baseline: 1.0002x; 1.0002x over previous
"""MoE CouncilLayer kernel for 8x TRN2 NeuronCores (expert-parallel, fp8).

Problem (all-expert MoE, B=2, T=1024, C=768, E=32, H=3072):
    gates = softmax(x @ gate_w + gate_b)                     # [N, E]
    h     = gelu(einsum('nc,ech->neh', x, w1) + b1)          # [N, E, H]
    y     = einsum('neh,ehc->nec', h, w2) + b2               # [N, E, C]
    out   = einsum('ne,nec->nc', gates, y)                   # [N, C]

Sharding: expert-parallel, 4 experts per core; x replicated. Each core
computes its 4 experts' gate-weighted partial sum; host adds the 8
partials.

All matmuls run as fp8e4 (e4m3) DoubleRow matmuls: each instruction
contracts two K=128 blocks at 0.5 cycles per output column - 4x the
fp16 MAC rate. Accuracy is recovered with Dekker-style 2-term fp8
splits of every operand (hi = q8(a), lo = q8(a - hi)) and a 3-product
scheme per logical matmul (hi*hi + lo*hi + hi*lo; the lo*lo term is
dropped), all accumulating in one fp32 PSUM group:
    mm1: h_psum = (x_hi + x_lo) @ w1_hi + x_hi @ w1_lo      (18 K128
         products per [128h x 512t] tile = 9 DoubleRow matmuls)
    mm2: y_psum = (hg_hi + hg_lo) @ w2_hi + hg_hi @ w2_lo
Weights are pre-scaled on the host before quantization (w1 x64,
w2 x128) so their sigma sits mid-range in e4m3 instead of at the
subnormal floor; the inverse scales fold into the ACT gelu input scale
and the gate constants (softmax 'ones' weights = 128 so the on-device
gates come out as g/128, cancelling mm2's x128 psum scale). Measured
end-to-end rel RMS error of this scheme is ~2e-3 (budget 2e-2); PE
time is 6/8 of the fp16 stream = ~740us vs the 983us fp16 floor.

Per-core dataflow per (token-group, expert):
    mm1 DR stream -> psum; ACT pass1 gelu->fp16 scratch, ACT pass2
    gelu->fp8 (hg_hi); DVE subtract -> hg_lo fp8. mm2 DR stream ->
    psum; Pool engine folds (psum + 128*b2) * g_bcast into the fp32
    yac accumulator (scalar_tensor_tensor + tensor_add), keeping DVE
    free for the lo-extraction.

Gates are computed on-device: logits via fp8 DoubleRow Dekker matmuls
(gw_hi/gw_lo x64 host-scaled, 3-product scheme -> ~0.1% logit error),
col-tiled per 512-token chunk in borrowed mm2 psum banks; fp32 ACT exp
(input scale 1/64, bias gate_b), fp16 ones-matmul denominators, DVE
reciprocal; [128, N] per-expert gate broadcast via a DRAM bounce.
Gate columns are permuted host-side so each core's 4 local experts sit
at columns 0..3 (keeps the SPMD program core-agnostic).

Head scheduling: the first four h-blocks of mm1 are traced
stream-outer/cc-outer across all 8 psum banks so PE consumes each
arriving x_hi/x_lo/w1_lo chunk as it lands; the gate softmax is traced
behind them and drains on ACT/DVE under the matmul stream.
"""

import numpy as np
import ml_dtypes

import concourse.tile as tile
from concourse import bacc, mybir
from concourse.bass_utils import run_bass_kernel_spmd

# Problem dims (hardcoded per harness contract)
B, T, C, E, H = 2, 1024, 768, 32, 3072
N = B * T  # 2048 tokens
NCORES = 8
EL = E // NCORES  # 4 local experts
CB = C // 128  # 6 c-blocks
HB = H // 128  # 24 h-blocks
CP = CB // 2  # 3 cc-pairs (DoubleRow K pairs)
HP = HB // 2  # 12 hb-pairs
TCG = 2  # token groups (1024 each)
TG = N // TCG  # 1024
TI = TG // 512  # 512-token chunks per group

S1 = 64.0  # host pre-scale on w1 (and gate_w) before fp8 quantization
S2 = 128.0  # host pre-scale on w2; also folded into softmax denominators

F8 = mybir.dt.float8e4
F16 = mybir.dt.float16
F32 = mybir.dt.float32
AF = mybir.ActivationFunctionType
DR = mybir.MatmulPerfMode.DoubleRow
NP8 = ml_dtypes.float8_e4m3

_CACHED_NC = None


def build_nc(act=AF.Gelu):
    nc = bacc.Bacc(trn_type="TRN2")

    xh_d = nc.dram_tensor("xh", [TCG, 128, CB * TG], F8, kind="ExternalInput")
    xl_d = nc.dram_tensor("xl", [TCG, 128, CB * TG], F8, kind="ExternalInput")
    gwh_d = nc.dram_tensor("gwh", [C, E], F8, kind="ExternalInput")
    gwl_d = nc.dram_tensor("gwl", [C, E], F8, kind="ExternalInput")
    gb_d = nc.dram_tensor("gb", [E, 1], F32, kind="ExternalInput")
    ones_d = nc.dram_tensor("ones32", [E, EL], F16, kind="ExternalInput")
    # w1/w2 are host-packed so every per-(expert, tile) DMA reads one
    # contiguous 3072B run per partition (<512B runs pay a 2x DMA penalty)
    w1h_d = nc.dram_tensor("w1h", [EL, HB // 4, 128, CB * 512], F8, kind="ExternalInput")
    w1l_d = nc.dram_tensor("w1l", [EL, HB // 4, 128, CB * 512], F8, kind="ExternalInput")
    b1_d = nc.dram_tensor("b1", [128, EL, HB], F32, kind="ExternalInput")
    w2h_d = nc.dram_tensor("w2h", [EL, CB, 128, HB * 128], F8, kind="ExternalInput")
    w2l_d = nc.dram_tensor("w2l", [EL, CB, 128, HB * 128], F8, kind="ExternalInput")
    b2P_d = nc.dram_tensor("b2P", [128, EL, CB], F32, kind="ExternalInput")
    outT_d = nc.dram_tensor("outT", [C, N], F32, kind="ExternalOutput")

    def w1_ap(dram, e, hbg):
        return dram[e, hbg, :, :].rearrange("p (cc h) -> p cc h", h=512)

    def w2_ap(dram, e, cb):
        return dram[e, cb, :, :].rearrange("p (hb c) -> p hb c", c=128)

    def x_ap(dram, tg):
        return dram[tg, :, :].rearrange("p (cc t) -> p cc t", t=TG)

    with tile.TileContext(nc) as tc:
        with (
            tc.tile_pool(name="const", bufs=1) as cp,
            tc.tile_pool(name="stream", bufs=1) as sp,
            tc.tile_pool(name="psum", bufs=1, space="PSUM") as pp,
            tc.tile_pool(name="dram", bufs=1, space="DRAM") as dp,
        ):
            # --- resident tiles ---
            xh_sb = cp.tile([128, CB, N], F8)
            xl_sb = cp.tile([128, CB, N], F8)
            gwh_sb = cp.tile([128, CB, E], F8)
            gwl_sb = cp.tile([128, CB, E], F8)
            gb_sb = cp.tile([E, 1], F32)
            ones_sb = cp.tile([E, EL], F16)
            b1_sb = cp.tile([128, EL, HB], F32)
            b2P_sb = cp.tile([128, EL, CB], F32)
            expT_sb = cp.tile([E, N], F32)
            expT16_sb = cp.tile([E, N], F16)
            g_bcast_sb = cp.tile([128, EL, N], F16)
            g_localT_sb = cp.tile([EL, N], F16)

            # DMA issue order = arrival order. The specially-traced first
            # mm1 block consumes, in order: w1h tile + x_hi (stream A,
            # cc-outer), x_lo (stream B), w1l tile (stream C). b1 rides
            # after the first x_hi chunk (needed by the first gelu); gate
            # consts next (softmax is traced right behind block 0); then
            # the second w1h/w1l tiles and the x halves for token-group 1.
            w1h_first = sp.tile([128, CB, 512], F8, tag="w1", bufs=6, name="w1t")
            w1l_first = sp.tile([128, CB, 512], F8, tag="w1", bufs=6, name="w1t")
            ccs = [slice(0, 2), slice(2, 4), slice(4, 6)]
            for ci, cs in enumerate(ccs):
                nc.sync.dma_start(w1h_first[:, cs, :], w1_ap(w1h_d, 0, 0)[:, cs, :])
                nc.sync.dma_start(xh_sb[:, cs, 0:TG], x_ap(xh_d, 0)[:, cs, :])
                if ci == 0:
                    nc.sync.dma_start(b1_sb, b1_d[:, :, :])
            nc.sync.dma_start(xl_sb[:, :, 0:TG], x_ap(xl_d, 0)[:, :, :])
            nc.sync.dma_start(w1l_first, w1_ap(w1l_d, 0, 0))
            nc.sync.dma_start(gwh_sb, gwh_d[:, :].rearrange("(cc p) e -> p cc e", p=128))
            nc.sync.dma_start(gwl_sb, gwl_d[:, :].rearrange("(cc p) e -> p cc e", p=128))
            nc.sync.dma_start(gb_sb, gb_d[:, :])
            nc.sync.dma_start(ones_sb, ones_d[:, :])
            # prefetch e0's second w1 tile pair ahead of the bulk second-half
            # x transfer so mm1 hbg1 isn't gated on it
            w1h_second = sp.tile([128, CB, 512], F8, tag="w1", bufs=6, name="w1t")
            w1l_second = sp.tile([128, CB, 512], F8, tag="w1", bufs=6, name="w1t")
            nc.sync.dma_start(w1h_second, w1_ap(w1h_d, 0, 1))
            nc.sync.dma_start(w1l_second, w1_ap(w1l_d, 0, 1))
            # token-group 1 x halves and b2P aren't needed for ~60us+; keep
            # them off the weight-streaming queue (Pool engine's queue)
            nc.gpsimd.dma_start(xh_sb[:, :, TG:N], x_ap(xh_d, 1)[:, :, :])
            nc.gpsimd.dma_start(xl_sb[:, :, TG:N], x_ap(xl_d, 1)[:, :, :])
            nc.gpsimd.dma_start(b2P_sb, b2P_d[:, :, :])

            def emit_softmax():
                # logits via fp8 DR Dekker matmuls into borrowed tag-"y"
                # psum slots (mm2 doesn't need them until ~130us in). Each
                # 512-token chunk gets its own psum bank at partition
                # offset 32*t4 (col-tiled so chunk groups run concurrently
                # on PE sub-arrays).
                lgs = [
                    pp.tile([128, 512], F32, tag="y", bufs=4, name="lg")
                    for _ in range(4)
                ]
                for t4 in range(4):
                    ts = slice(t4 * 512, (t4 + 1) * 512)
                    out = lgs[t4][0:32, :]
                    n_st = 3 * CP
                    i = 0
                    for cpi in range(CP):
                        cs = slice(2 * cpi, 2 * cpi + 2)
                        for gw_t, x_t in (
                            (gwh_sb, xh_sb),
                            (gwh_sb, xl_sb),
                            (gwl_sb, xh_sb),
                        ):
                            nc.tensor.matmul(
                                out,
                                gw_t[:, cs, :],
                                x_t[:, cs, ts],
                                start=(i == 0),
                                stop=(i == n_st - 1),
                                perf_mode=DR,
                            )
                            i += 1
                for t4 in range(N // 512):
                    ts = slice(t4 * 512, (t4 + 1) * 512)
                    lgs4 = lgs[t4][0:32, :]
                    nc.scalar.activation(
                        expT_sb[:, ts], lgs4, AF.Exp, bias=gb_sb, scale=1.0 / S1
                    )
                    nc.scalar.activation(
                        expT16_sb[:, ts], lgs4, AF.Exp, bias=gb_sb, scale=1.0 / S1
                    )
                # denominators (x S2 via the ones constant): second pass so
                # the exps have drained on ACT by the time PE reaches these
                dns = [
                    pp.tile([128, 512], F32, tag="y", bufs=4, name="dn")
                    for _ in range(4)
                ]
                for t4 in range(N // 512):
                    ts = slice(t4 * 512, (t4 + 1) * 512)
                    nc.tensor.matmul(
                        dns[t4][32 * t4 : 32 * t4 + EL, :],
                        ones_sb[:, :],
                        expT16_sb[:, ts],
                        start=True,
                        stop=True,
                        tile_position=(0, 32 * t4),
                    )
                for t4 in range(N // 512):
                    ts = slice(t4 * 512, (t4 + 1) * 512)
                    rc = sp.tile([EL, 512], F32, tag="recip", bufs=2, name="rc")
                    nc.vector.reciprocal(rc, dns[t4][32 * t4 : 32 * t4 + EL, :])
                    nc.vector.tensor_mul(g_localT_sb[:, ts], expT_sb[0:EL, ts], rc)
                g_dram = dp.tile([EL, N], F16, name="g_dram")
                nc.gpsimd.dma_start(g_dram, g_localT_sb[:, :])
                for j in range(EL):
                    nc.gpsimd.dma_start(
                        g_bcast_sb[:, j, :],
                        g_dram[j : j + 1, :].to_broadcast((128, N)),
                    )

            def emit_gelu_split(e, hb, hps, hgh, hgl):
                # psum -> fp16 scratch + fp8 hi (ACT), fp8 lo (DVE)
                for ti in range(TI):
                    lts = slice(ti * 512, (ti + 1) * 512)
                    hf = sp.tile([128, 512], F16, tag="hf", bufs=4, name="hf")
                    bias = b1_sb[:, e, hb : hb + 1]
                    nc.scalar.activation(hf, hps[ti], act, bias=bias, scale=1.0 / S1)
                    nc.scalar.activation(
                        hgh[:, hb, lts], hps[ti], act, bias=bias, scale=1.0 / S1
                    )
                    nc.vector.tensor_tensor(
                        hgl[:, hb, lts], hf, hgh[:, hb, lts], mybir.AluOpType.subtract
                    )

            # mm1 stream/cc-pair schedule: the x_hi@w1_lo correction is
            # dropped on cc-pair 0 (K[0:256]) - residual 2.75%/sqrt(3) =
            # 1.6% rel RMS (mirror-verified 0.01605 end to end vs the 2e-2
            # gate) - saving 1 of 9 DR matmuls per mm1 tile (~41us).
            def mm1_pairs(w1h_t, w1l_t):
                out = []
                for si, (w_t, x_t) in enumerate(
                    ((w1h_t, xh_sb), (w1h_t, xl_sb), (w1l_t, xh_sb))
                ):
                    for cpi in range(CP):
                        if si == 2 and cpi == 0:
                            continue
                        out.append((si, cpi, w_t, x_t))
                return out

            def emit_mm1_first(hgh, hgl):
                # first 4 h-blocks of (tg0, e0), traced stream-outer and
                # cc-outer across all 8 psum banks: PE's in-order stream
                # consumes each arriving x_hi chunk, then x_lo, then w1l.
                hps8 = [
                    [
                        pp.tile(
                            [128, 512],
                            F32,
                            tag=("h" if hbi < 2 else "y"),
                            bufs=4,
                            name="hps",
                        )
                        for _ in range(TI)
                    ]
                    for hbi in range(4)
                ]
                pairs = mm1_pairs(w1h_first, w1l_first)
                for pi, (si, cpi, w_t, x_t) in enumerate(pairs):
                    cs = slice(2 * cpi, 2 * cpi + 2)
                    for ti in range(TI):
                        for hbi in range(4):
                            nc.tensor.matmul(
                                hps8[hbi][ti],
                                w_t[:, cs, hbi * 128 : (hbi + 1) * 128],
                                x_t[:, cs, ti * 512 : (ti + 1) * 512],
                                start=(pi == 0),
                                stop=(pi == len(pairs) - 1),
                                perf_mode=DR,
                            )
                for hbi in range(4):
                    emit_gelu_split(0, hbi, hps8[hbi], hgh, hgl)

            def emit_mm1(tg, e, hgh, hgl, hbg_start=0, hbg_end=HB // 4, pre=None):
                # mm1: h_psum = (x_hi + x_lo) @ w1_hi + x_hi @ w1_lo[K256:]
                for hbg in range(hbg_start, hbg_end):
                    if tg == 0 and e == 0 and hbg == 1:
                        w1h_t, w1l_t = w1h_second, w1l_second
                    elif hbg == hbg_start and pre is not None:
                        w1h_t, w1l_t = pre
                    else:
                        w1h_t = sp.tile([128, CB, 512], F8, tag="w1", bufs=6, name="w1t")
                        nc.sync.dma_start(w1h_t, w1_ap(w1h_d, e, hbg))
                        w1l_t = sp.tile([128, CB, 512], F8, tag="w1", bufs=6, name="w1t")
                        nc.sync.dma_start(w1l_t, w1_ap(w1l_d, e, hbg))
                    pairs = mm1_pairs(w1h_t, w1l_t)
                    for hbi in range(4):
                        hb = hbg * 4 + hbi
                        hps = [
                            pp.tile([128, 512], F32, tag="h", bufs=4, name="hps")
                            for _ in range(TI)
                        ]
                        for ti in range(TI):
                            gts = slice(tg * TG + ti * 512, tg * TG + (ti + 1) * 512)
                            for pi, (si, cpi, w_t, x_t) in enumerate(pairs):
                                cs = slice(2 * cpi, 2 * cpi + 2)
                                nc.tensor.matmul(
                                    hps[ti],
                                    w_t[:, cs, hbi * 128 : (hbi + 1) * 128],
                                    x_t[:, cs, gts],
                                    start=(pi == 0),
                                    stop=(pi == len(pairs) - 1),
                                    perf_mode=DR,
                                )
                        emit_gelu_split(e, hb, hps, hgh, hgl)

            def emit_mm2(tg, e, hgh, hgl, yac):
                # mm2: y_psum = (hg_hi + hg_lo) @ w2_hi + hg_hi @ w2_lo;
                # then yac (+)= (y_psum + S2*b2) * g_bcast: the psum-reading
                # stt on DVE (GPSIMD can't access PSUM), the SBUF-only
                # cross-expert add on the otherwise-idle Pool engine
                for cb in range(CB):
                    w2h_t = sp.tile([128, HB, 128], F8, tag="w2", bufs=8, name="w2t")
                    w2l_t = sp.tile([128, HB, 128], F8, tag="w2", bufs=8, name="w2t")
                    nc.sync.dma_start(w2h_t, w2_ap(w2h_d, e, cb))
                    nc.sync.dma_start(w2l_t, w2_ap(w2l_d, e, cb))
                    yps = [
                        pp.tile([128, 512], F32, tag="y", bufs=4, name="yps")
                        for _ in range(TI)
                    ]
                    for ti in range(TI):
                        lts = slice(ti * 512, (ti + 1) * 512)
                        i = 0
                        for hpi in range(HP):
                            hs = slice(2 * hpi, 2 * hpi + 2)
                            for w_t, h_t in (
                                (w2h_t, hgh),
                                (w2h_t, hgl),
                                (w2l_t, hgh),
                            ):
                                nc.tensor.matmul(
                                    yps[ti],
                                    w_t[:, hs, :],
                                    h_t[:, hs, lts],
                                    start=(i == 0),
                                    stop=(i == 3 * HP - 1),
                                    perf_mode=DR,
                                )
                                i += 1
                    for ti in range(TI):
                        gts = slice(tg * TG + ti * 512, tg * TG + (ti + 1) * 512)
                        lts = slice(ti * 512, (ti + 1) * 512)
                        if e == 0:
                            nc.vector.scalar_tensor_tensor(
                                out=yac[:, cb, lts],
                                in0=yps[ti],
                                scalar=b2P_sb[:, e, cb : cb + 1],
                                in1=g_bcast_sb[:, e, gts],
                                op0=mybir.AluOpType.add,
                                op1=mybir.AluOpType.mult,
                            )
                        else:
                            yt = sp.tile([128, 512], F32, tag="ytmp", bufs=2, name="yt")
                            nc.vector.scalar_tensor_tensor(
                                out=yt,
                                in0=yps[ti],
                                scalar=b2P_sb[:, e, cb : cb + 1],
                                in1=g_bcast_sb[:, e, gts],
                                op0=mybir.AluOpType.add,
                                op1=mybir.AluOpType.mult,
                            )
                            nc.gpsimd.tensor_tensor(
                                yac[:, cb, lts], yt, yac[:, cb, lts], mybir.AluOpType.add
                            )

            # --- main. Trace order = PE order: the special first block
            # (fills the x/w1 arrival window), the gate prologue (drains on
            # ACT/DVE under the matmul stream), then the expert stream.
            prefetched = {}
            for tg in range(TCG):
                hgh = sp.tile([128, HB, TG], F8, tag="hgh", bufs=1, name="hgh")
                hgl = sp.tile([128, HB, TG], F8, tag="hgl", bufs=1, name="hgl")
                yac = sp.tile([128, CB, TG], F32, tag="yacc", bufs=1, name="yac")
                for e in range(EL):
                    if tg == 0 and e == 0:
                        emit_mm1_first(hgh, hgl)
                        emit_softmax()
                        emit_mm1(tg, e, hgh, hgl, hbg_start=1)
                    else:
                        emit_mm1(tg, e, hgh, hgl, pre=prefetched.pop((tg, e), None))
                    # prefetch the next block's first w1 tile pair ahead of
                    # mm2's w2 DMA burst so the expert boundary doesn't stall
                    ntg, ne = (tg, e + 1) if e + 1 < EL else (tg + 1, 0)
                    if ntg < TCG:
                        w1h_p = sp.tile([128, CB, 512], F8, tag="w1", bufs=6, name="w1t")
                        nc.sync.dma_start(w1h_p, w1_ap(w1h_d, ne, 0))
                        w1l_p = sp.tile([128, CB, 512], F8, tag="w1", bufs=6, name="w1t")
                        nc.sync.dma_start(w1l_p, w1_ap(w1l_d, ne, 0))
                        prefetched[(ntg, ne)] = (w1h_p, w1l_p)
                    emit_mm2(tg, e, hgh, hgl, yac)
                for cb in range(CB):
                    for ti in range(TI):
                        nc.gpsimd.dma_start(
                            outT_d[
                                cb * 128 : (cb + 1) * 128,
                                tg * TG + ti * 512 : tg * TG + (ti + 1) * 512,
                            ],
                            yac[:, cb, ti * 512 : (ti + 1) * 512],
                        )

    nc.compile()
    return nc


def _get_nc():
    global _CACHED_NC
    if _CACHED_NC is None:
        _CACHED_NC = build_nc()
    return _CACHED_NC


def _split8(a, scale):
    """Dekker 2-term fp8 split of scale*a. Returns (hi, lo) as e4m3."""
    s = (a * scale).astype(np.float32)
    hi = s.astype(NP8)
    lo = (s - hi.astype(np.float32)).astype(NP8)
    return hi, lo


def make_in_maps(x, gate_w, gate_b, w1, b1, w2, b2):
    x = np.asarray(x, np.float32)
    gate_w = np.asarray(gate_w, np.float32)
    gate_b = np.asarray(gate_b, np.float32)
    w1 = np.asarray(w1, np.float32)
    b1 = np.asarray(b1, np.float32)
    w2 = np.asarray(w2, np.float32)
    b2 = np.asarray(b2, np.float32)

    # pack x as [tg, p, cc*TG]: contiguous 1KB+ runs per partition
    xT = x.reshape(N, C).T.reshape(CB, 128, TCG, TG).transpose(2, 1, 0, 3)
    xT = np.ascontiguousarray(xT).reshape(TCG, 128, CB * TG)
    xh, xl = _split8(xT, 1.0)

    ones32 = np.full((E, EL), S2, np.float16)

    in_maps = []
    for i in range(NCORES):
        lo, hi = EL * i, EL * (i + 1)
        perm = list(range(lo, hi)) + [e for e in range(E) if not (lo <= e < hi)]
        gwh, gwl = _split8(np.ascontiguousarray(gate_w[:, perm]), S1)
        # w1 packed [e, hbg, p, cc*512]; w2 packed [e, cb, p, hb*128]
        w1p = w1[lo:hi].reshape(EL, CB, 128, HB // 4, 512).transpose(0, 3, 2, 1, 4)
        w1p = np.ascontiguousarray(w1p).reshape(EL, HB // 4, 128, CB * 512)
        w2p = w2[lo:hi].reshape(EL, HB, 128, CB, 128).transpose(0, 3, 2, 1, 4)
        w2p = np.ascontiguousarray(w2p).reshape(EL, CB, 128, HB * 128)
        w1h, w1l = _split8(w1p, S1)
        w2h, w2l = _split8(w2p, S2)
        in_maps.append(
            {
                "xh": xh,
                "xl": xl,
                "gwh": gwh,
                "gwl": gwl,
                "gb": np.ascontiguousarray(gate_b[perm]).reshape(E, 1),
                "ones32": ones32,
                "w1h": w1h,
                "w1l": w1l,
                "b1": np.ascontiguousarray(
                    b1[lo:hi].reshape(EL, HB, 128).transpose(2, 0, 1)
                ),
                "w2h": w2h,
                "w2l": w2l,
                "b2P": np.ascontiguousarray(
                    (S2 * b2[lo:hi]).reshape(EL, CB, 128).transpose(2, 0, 1)
                ),
            }
        )
    return in_maps


def kernel(x, gate_w, gate_b, w1, b1, w2, b2, _trace=False, _tmpdir=None):
    nc = _get_nc()
    in_maps = make_in_maps(x, gate_w, gate_b, w1, b1, w2, b2)
    res = run_bass_kernel_spmd(
        nc,
        in_maps,
        core_ids=list(range(NCORES)),
        trace=_trace,
        tmpdir=_tmpdir,
    )
    acc = res.results[0]["outT"].astype(np.float64)
    for r in res.results[1:]:
        acc += r["outT"]
    out = acc.T.reshape(B, T, C).astype(np.float32)
    if _trace:
        kernel._last_results = res
    return out


# revision 16
# speedup vs baseline: 1.0052x; 1.0049x over previous
"""MoE CouncilLayer kernel for 8x TRN2 NeuronCores (expert-parallel, fp8).

Problem (all-expert MoE, B=2, T=1024, C=768, E=32, H=3072):
    gates = softmax(x @ gate_w + gate_b)                     # [N, E]
    h     = gelu(einsum('nc,ech->neh', x, w1) + b1)          # [N, E, H]
    y     = einsum('neh,ehc->nec', h, w2) + b2               # [N, E, C]
    out   = einsum('ne,nec->nc', gates, y)                   # [N, C]

Sharding: expert-parallel, 4 experts per core; x replicated. Each core
computes its 4 experts' gate-weighted partial sum; host adds the 8
partials.

All matmuls run as fp8e4 (e4m3) DoubleRow matmuls: each instruction
contracts two K=128 blocks at 0.5 cycles per output column - 4x the
fp16 MAC rate. Accuracy is recovered with Dekker-style 2-term fp8
splits of every operand (hi = q8(a), lo = q8(a - hi)) and a 3-product
scheme per logical matmul (hi*hi + lo*hi + hi*lo; the lo*lo term is
dropped), all accumulating in one fp32 PSUM group:
    mm1: h_psum = (x_hi + x_lo) @ w1_hi + x_hi @ w1_lo      (18 K128
         products per [128h x 512t] tile = 9 DoubleRow matmuls)
    mm2: y_psum = (hg_hi + hg_lo) @ w2_hi + hg_hi @ w2_lo
Weights are pre-scaled on the host before quantization (w1 x64,
w2 x128) so their sigma sits mid-range in e4m3 instead of at the
subnormal floor; the inverse scales fold into the ACT gelu input scale
and the gate constants (softmax 'ones' weights = 128 so the on-device
gates come out as g/128, cancelling mm2's x128 psum scale). Measured
end-to-end rel RMS error of this scheme is ~2e-3 (budget 2e-2); PE
time is 6/8 of the fp16 stream = ~740us vs the 983us fp16 floor.

Per-core dataflow per (token-group, expert):
    mm1 DR stream -> psum; ACT pass1 gelu->fp16 scratch, ACT pass2
    gelu->fp8 (hg_hi); DVE subtract -> hg_lo fp8. mm2 DR stream ->
    psum; Pool engine folds (psum + 128*b2) * g_bcast into the fp32
    yac accumulator (scalar_tensor_tensor + tensor_add), keeping DVE
    free for the lo-extraction.

Gates are computed on-device: logits via fp8 DoubleRow Dekker matmuls
(gw_hi/gw_lo x64 host-scaled, 3-product scheme -> ~0.1% logit error),
col-tiled per 512-token chunk in borrowed mm2 psum banks; fp32 ACT exp
(input scale 1/64, bias gate_b), fp16 ones-matmul denominators, DVE
reciprocal; [128, N] per-expert gate broadcast via a DRAM bounce.
Gate columns are permuted host-side so each core's 4 local experts sit
at columns 0..3 (keeps the SPMD program core-agnostic).

Head scheduling: the first four h-blocks of mm1 are traced
stream-outer/cc-outer across all 8 psum banks so PE consumes each
arriving x_hi/x_lo/w1_lo chunk as it lands; the gate softmax is traced
behind them and drains on ACT/DVE under the matmul stream.
"""

import numpy as np
import ml_dtypes

import concourse.tile as tile
from concourse import bacc, mybir
from concourse.bass_utils import run_bass_kernel_spmd

# Problem dims (hardcoded per harness contract)
B, T, C, E, H = 2, 1024, 768, 32, 3072
N = B * T  # 2048 tokens
NCORES = 8
EL = E // NCORES  # 4 local experts
CB = C // 128  # 6 c-blocks
HB = H // 128  # 24 h-blocks
CP = CB // 2  # 3 cc-pairs (DoubleRow K pairs)
HP = HB // 2  # 12 hb-pairs
TCG = 2  # token groups (1024 each)
TG = N // TCG  # 1024
TI = TG // 512  # 512-token chunks per group

S1 = 64.0  # host pre-scale on w1 (and gate_w) before fp8 quantization
S2 = 128.0  # host pre-scale on w2; also folded into softmax denominators

F8 = mybir.dt.float8e4
F16 = mybir.dt.float16
F32 = mybir.dt.float32
AF = mybir.ActivationFunctionType
DR = mybir.MatmulPerfMode.DoubleRow
NP8 = ml_dtypes.float8_e4m3

_CACHED_NC = None


def build_nc(act=AF.Gelu):
    nc = bacc.Bacc(trn_type="TRN2")

    xh_d = nc.dram_tensor("xh", [TCG, 128, CB * TG], F8, kind="ExternalInput")
    xl_d = nc.dram_tensor("xl", [TCG, 128, CB * TG], F8, kind="ExternalInput")
    gwh_d = nc.dram_tensor("gwh", [C, E], F8, kind="ExternalInput")
    gwl_d = nc.dram_tensor("gwl", [C, E], F8, kind="ExternalInput")
    gb_d = nc.dram_tensor("gb", [E, 1], F32, kind="ExternalInput")
    ones_d = nc.dram_tensor("ones32", [E, EL], F16, kind="ExternalInput")
    # w1/w2 are host-packed so every per-(expert, tile) DMA reads one
    # contiguous 3072B run per partition (<512B runs pay a 2x DMA penalty)
    w1h_d = nc.dram_tensor("w1h", [EL, HB // 4, 128, CB * 512], F8, kind="ExternalInput")
    w1l_d = nc.dram_tensor("w1l", [EL, HB // 4, 128, CB * 512], F8, kind="ExternalInput")
    b1_d = nc.dram_tensor("b1", [128, EL, HB], F32, kind="ExternalInput")
    w2h_d = nc.dram_tensor("w2h", [EL, CB, 128, HB * 128], F8, kind="ExternalInput")
    w2l_d = nc.dram_tensor("w2l", [EL, CB, 128, HB * 128], F8, kind="ExternalInput")
    b2P_d = nc.dram_tensor("b2P", [128, EL, CB], F32, kind="ExternalInput")
    outT_d = nc.dram_tensor("outT", [C, N], F32, kind="ExternalOutput")

    def w1_ap(dram, e, hbg):
        return dram[e, hbg, :, :].rearrange("p (cc h) -> p cc h", h=512)

    def w2_ap(dram, e, cb):
        return dram[e, cb, :, :].rearrange("p (hb c) -> p hb c", c=128)

    def x_ap(dram, tg):
        return dram[tg, :, :].rearrange("p (cc t) -> p cc t", t=TG)

    with tile.TileContext(nc) as tc:
        with (
            tc.tile_pool(name="const", bufs=1) as cp,
            tc.tile_pool(name="stream", bufs=1) as sp,
            tc.tile_pool(name="psum", bufs=1, space="PSUM") as pp,
            tc.tile_pool(name="dram", bufs=1, space="DRAM") as dp,
        ):
            # --- resident tiles ---
            xh_sb = cp.tile([128, CB, N], F8)
            xl_sb = cp.tile([128, CB, N], F8)
            gwh_sb = cp.tile([128, CB, E], F8)
            gwl_sb = cp.tile([128, CB, E], F8)
            gb_sb = cp.tile([E, 1], F32)
            ones_sb = cp.tile([E, EL], F16)
            b1_sb = cp.tile([128, EL, HB], F32)
            b2P_sb = cp.tile([128, EL, CB], F32)
            expT_sb = cp.tile([E, N], F32)
            expT16_sb = cp.tile([E, N], F16)
            g_bcast_sb = cp.tile([128, EL, N], F16)
            g_localT_sb = cp.tile([EL, N], F16)

            # DMA issue order = arrival order. The specially-traced first
            # mm1 block consumes, in order: w1h tile + x_hi (stream A,
            # cc-outer), x_lo (stream B), w1l tile (stream C). b1 rides
            # after the first x_hi chunk (needed by the first gelu); gate
            # consts next (softmax is traced right behind block 0); then
            # the second w1h/w1l tiles and the x halves for token-group 1.
            w1h_first = sp.tile([128, CB, 512], F8, tag="w1", bufs=6, name="w1t")
            w1l_first = sp.tile([128, CB, 512], F8, tag="w1", bufs=6, name="w1t")
            ccs = [slice(0, 2), slice(2, 4), slice(4, 6)]
            for ci, cs in enumerate(ccs):
                nc.sync.dma_start(w1h_first[:, cs, :], w1_ap(w1h_d, 0, 0)[:, cs, :])
                nc.sync.dma_start(xh_sb[:, cs, 0:TG], x_ap(xh_d, 0)[:, cs, :])
                if ci == 0:
                    nc.sync.dma_start(b1_sb, b1_d[:, :, :])
            nc.sync.dma_start(xl_sb[:, :, 0:TG], x_ap(xl_d, 0)[:, :, :])
            nc.sync.dma_start(w1l_first, w1_ap(w1l_d, 0, 0))
            nc.sync.dma_start(gwh_sb, gwh_d[:, :].rearrange("(cc p) e -> p cc e", p=128))
            nc.sync.dma_start(gwl_sb, gwl_d[:, :].rearrange("(cc p) e -> p cc e", p=128))
            nc.sync.dma_start(gb_sb, gb_d[:, :])
            nc.sync.dma_start(ones_sb, ones_d[:, :])
            # prefetch e0's second w1 tile pair ahead of the bulk second-half
            # x transfer so mm1 hbg1 isn't gated on it
            w1h_second = sp.tile([128, CB, 512], F8, tag="w1", bufs=6, name="w1t")
            w1l_second = sp.tile([128, CB, 512], F8, tag="w1", bufs=6, name="w1t")
            nc.sync.dma_start(w1h_second, w1_ap(w1h_d, 0, 1))
            nc.sync.dma_start(w1l_second, w1_ap(w1l_d, 0, 1))


            def emit_softmax():
                # logits via fp8 DR Dekker matmuls into borrowed tag-"y"
                # psum slots (mm2 doesn't need them until ~130us in). Each
                # 512-token chunk gets its own psum bank at partition
                # offset 32*t4 (col-tiled so chunk groups run concurrently
                # on PE sub-arrays).
                lgs = [
                    pp.tile([128, 512], F32, tag="y", bufs=4, name="lg")
                    for _ in range(4)
                ]
                for t4 in range(4):
                    ts = slice(t4 * 512, (t4 + 1) * 512)
                    out = lgs[t4][0:32, :]
                    n_st = 3 * CP
                    i = 0
                    for cpi in range(CP):
                        cs = slice(2 * cpi, 2 * cpi + 2)
                        for gw_t, x_t in (
                            (gwh_sb, xh_sb),
                            (gwh_sb, xl_sb),
                            (gwl_sb, xh_sb),
                        ):
                            nc.tensor.matmul(
                                out,
                                gw_t[:, cs, :],
                                x_t[:, cs, ts],
                                start=(i == 0),
                                stop=(i == n_st - 1),
                                perf_mode=DR,
                            )
                            i += 1
                for t4 in range(N // 512):
                    ts = slice(t4 * 512, (t4 + 1) * 512)
                    lgs4 = lgs[t4][0:32, :]
                    nc.scalar.activation(
                        expT_sb[:, ts], lgs4, AF.Exp, bias=gb_sb, scale=1.0 / S1
                    )
                    nc.scalar.activation(
                        expT16_sb[:, ts], lgs4, AF.Exp, bias=gb_sb, scale=1.0 / S1
                    )
                # denominators (x S2 via the ones constant): second pass so
                # the exps have drained on ACT by the time PE reaches these
                dns = [
                    pp.tile([128, 512], F32, tag="y", bufs=4, name="dn")
                    for _ in range(4)
                ]
                for t4 in range(N // 512):
                    ts = slice(t4 * 512, (t4 + 1) * 512)
                    nc.tensor.matmul(
                        dns[t4][32 * t4 : 32 * t4 + EL, :],
                        ones_sb[:, :],
                        expT16_sb[:, ts],
                        start=True,
                        stop=True,
                        tile_position=(0, 32 * t4),
                    )
                for t4 in range(N // 512):
                    ts = slice(t4 * 512, (t4 + 1) * 512)
                    rc = sp.tile([EL, 512], F32, tag="recip", bufs=2, name="rc")
                    nc.vector.reciprocal(rc, dns[t4][32 * t4 : 32 * t4 + EL, :])
                    nc.vector.tensor_mul(g_localT_sb[:, ts], expT_sb[0:EL, ts], rc)
                g_dram = dp.tile([EL, N], F16, name="g_dram")
                nc.gpsimd.dma_start(g_dram, g_localT_sb[:, :])
                for j in range(EL):
                    nc.gpsimd.dma_start(
                        g_bcast_sb[:, j, :],
                        g_dram[j : j + 1, :].to_broadcast((128, N)),
                    )

            def emit_gelu_split(e, hb, hps, hgh, hgl):
                # psum -> fp16 scratch + fp8 hi (ACT), fp8 lo (DVE)
                for ti in range(TI):
                    lts = slice(ti * 512, (ti + 1) * 512)
                    hf = sp.tile([128, 512], F16, tag="hf", bufs=4, name="hf")
                    bias = b1_sb[:, e, hb : hb + 1]
                    nc.scalar.activation(hf, hps[ti], act, bias=bias, scale=1.0 / S1)
                    nc.scalar.activation(
                        hgh[:, hb, lts], hps[ti], act, bias=bias, scale=1.0 / S1
                    )
                    nc.vector.tensor_tensor(
                        hgl[:, hb, lts], hf, hgh[:, hb, lts], mybir.AluOpType.subtract
                    )

            # mm1 stream/cc-pair schedule: the x_hi@w1_lo correction is
            # dropped on cc-pair 0 (K[0:256]) - residual 2.75%/sqrt(3) =
            # 1.6% rel RMS (mirror-verified 0.01605 end to end vs the 2e-2
            # gate) - saving 1 of 9 DR matmuls per mm1 tile (~41us).
            def mm1_pairs(w1h_t, w1l_t):
                out = []
                for si, (w_t, x_t) in enumerate(
                    ((w1h_t, xh_sb), (w1h_t, xl_sb), (w1l_t, xh_sb))
                ):
                    for cpi in range(CP):
                        if si == 2 and cpi == 0:
                            continue
                        out.append((si, cpi, w_t, x_t))
                return out

            def emit_mm1_first(hgh, hgl):
                # first 4 h-blocks of (tg0, e0), traced stream-outer and
                # cc-outer across all 8 psum banks: PE's in-order stream
                # consumes each arriving x_hi chunk, then x_lo, then w1l.
                hps8 = [
                    [
                        pp.tile(
                            [128, 512],
                            F32,
                            tag=("h" if hbi < 2 else "y"),
                            bufs=4,
                            name="hps",
                        )
                        for _ in range(TI)
                    ]
                    for hbi in range(4)
                ]
                pairs = mm1_pairs(w1h_first, w1l_first)
                for pi, (si, cpi, w_t, x_t) in enumerate(pairs):
                    cs = slice(2 * cpi, 2 * cpi + 2)
                    for ti in range(TI):
                        for hbi in range(4):
                            nc.tensor.matmul(
                                hps8[hbi][ti],
                                w_t[:, cs, hbi * 128 : (hbi + 1) * 128],
                                x_t[:, cs, ti * 512 : (ti + 1) * 512],
                                start=(pi == 0),
                                stop=(pi == len(pairs) - 1),
                                perf_mode=DR,
                            )
                for hbi in range(4):
                    emit_gelu_split(0, hbi, hps8[hbi], hgh, hgl)

            def emit_mm1(tg, e, hgh, hgl, hbg_start=0, hbg_end=HB // 4, pre=None):
                # mm1: h_psum = (x_hi + x_lo) @ w1_hi + x_hi @ w1_lo[K256:]
                for hbg in range(hbg_start, hbg_end):
                    if tg == 0 and e == 0 and hbg == 1:
                        w1h_t, w1l_t = w1h_second, w1l_second
                    elif hbg == hbg_start and pre is not None:
                        w1h_t, w1l_t = pre
                    else:
                        w1h_t = sp.tile([128, CB, 512], F8, tag="w1", bufs=6, name="w1t")
                        nc.sync.dma_start(w1h_t, w1_ap(w1h_d, e, hbg))
                        w1l_t = sp.tile([128, CB, 512], F8, tag="w1", bufs=6, name="w1t")
                        nc.sync.dma_start(w1l_t, w1_ap(w1l_d, e, hbg))
                    pairs = mm1_pairs(w1h_t, w1l_t)
                    for hbi in range(4):
                        hb = hbg * 4 + hbi
                        hps = [
                            pp.tile([128, 512], F32, tag="h", bufs=4, name="hps")
                            for _ in range(TI)
                        ]
                        for ti in range(TI):
                            gts = slice(tg * TG + ti * 512, tg * TG + (ti + 1) * 512)
                            for pi, (si, cpi, w_t, x_t) in enumerate(pairs):
                                cs = slice(2 * cpi, 2 * cpi + 2)
                                nc.tensor.matmul(
                                    hps[ti],
                                    w_t[:, cs, hbi * 128 : (hbi + 1) * 128],
                                    x_t[:, cs, gts],
                                    start=(pi == 0),
                                    stop=(pi == len(pairs) - 1),
                                    perf_mode=DR,
                                )
                        emit_gelu_split(e, hb, hps, hgh, hgl)

            def emit_mm2(tg, e, hgh, hgl, yac):
                # mm2: y_psum = (hg_hi + hg_lo) @ w2_hi + hg_hi @ w2_lo;
                # then yac (+)= (y_psum + S2*b2) * g_bcast: the psum-reading
                # stt on DVE (GPSIMD can't access PSUM), the SBUF-only
                # cross-expert add on the otherwise-idle Pool engine
                for cb in range(CB):
                    w2h_t = sp.tile([128, HB, 128], F8, tag="w2", bufs=8, name="w2t")
                    w2l_t = sp.tile([128, HB, 128], F8, tag="w2", bufs=8, name="w2t")
                    nc.sync.dma_start(w2h_t, w2_ap(w2h_d, e, cb))
                    nc.sync.dma_start(w2l_t, w2_ap(w2l_d, e, cb))
                    yps = [
                        pp.tile([128, 512], F32, tag="y", bufs=4, name="yps")
                        for _ in range(TI)
                    ]
                    for ti in range(TI):
                        lts = slice(ti * 512, (ti + 1) * 512)
                        i = 0
                        for hpi in range(HP):
                            hs = slice(2 * hpi, 2 * hpi + 2)
                            for w_t, h_t in (
                                (w2h_t, hgh),
                                (w2h_t, hgl),
                                (w2l_t, hgh),
                            ):
                                nc.tensor.matmul(
                                    yps[ti],
                                    w_t[:, hs, :],
                                    h_t[:, hs, lts],
                                    start=(i == 0),
                                    stop=(i == 3 * HP - 1),
                                    perf_mode=DR,
                                )
                                i += 1
                    for ti in range(TI):
                        gts = slice(tg * TG + ti * 512, tg * TG + (ti + 1) * 512)
                        lts = slice(ti * 512, (ti + 1) * 512)
                        if e == 0:
                            nc.vector.scalar_tensor_tensor(
                                out=yac[:, cb, lts],
                                in0=yps[ti],
                                scalar=b2P_sb[:, e, cb : cb + 1],
                                in1=g_bcast_sb[:, e, gts],
                                op0=mybir.AluOpType.add,
                                op1=mybir.AluOpType.mult,
                            )
                        else:
                            yt = sp.tile([128, 512], F32, tag="ytmp", bufs=2, name="yt")
                            nc.vector.scalar_tensor_tensor(
                                out=yt,
                                in0=yps[ti],
                                scalar=b2P_sb[:, e, cb : cb + 1],
                                in1=g_bcast_sb[:, e, gts],
                                op0=mybir.AluOpType.add,
                                op1=mybir.AluOpType.mult,
                            )
                            nc.gpsimd.tensor_tensor(
                                yac[:, cb, lts], yt, yac[:, cb, lts], mybir.AluOpType.add
                            )

            # --- main. Trace order = PE order: the special first block
            # (fills the x/w1 arrival window), the gate prologue (drains on
            # ACT/DVE under the matmul stream), then the expert stream.
            prefetched = {}
            for tg in range(TCG):
                hgh = sp.tile([128, HB, TG], F8, tag="hgh", bufs=1, name="hgh")
                hgl = sp.tile([128, HB, TG], F8, tag="hgl", bufs=1, name="hgl")
                yac = sp.tile([128, CB, TG], F32, tag="yacc", bufs=1, name="yac")
                for e in range(EL):
                    if tg == 0 and e == 0:
                        emit_mm1_first(hgh, hgl)
                        emit_softmax()
                        emit_mm1(tg, e, hgh, hgl, hbg_start=1)
                    else:
                        emit_mm1(tg, e, hgh, hgl, pre=prefetched.pop((tg, e), None))
                    if tg == 0 and e == 0:
                        # tg1 x halves + b2P: needed from ~60us (b2P) / ~370us
                        # (x); issued behind e0's full w1 stream so they don't
                        # delay the head, but before the w2/prefetch bursts
                        nc.sync.dma_start(xh_sb[:, :, TG:N], x_ap(xh_d, 1)[:, :, :])
                        nc.sync.dma_start(xl_sb[:, :, TG:N], x_ap(xl_d, 1)[:, :, :])
                        nc.sync.dma_start(b2P_sb, b2P_d[:, :, :])
                    # prefetch the next block's first w1 tile pair ahead of
                    # mm2's w2 DMA burst so the expert boundary doesn't stall
                    ntg, ne = (tg, e + 1) if e + 1 < EL else (tg + 1, 0)
                    if ntg < TCG:
                        w1h_p = sp.tile([128, CB, 512], F8, tag="w1", bufs=6, name="w1t")
                        nc.sync.dma_start(w1h_p, w1_ap(w1h_d, ne, 0))
                        w1l_p = sp.tile([128, CB, 512], F8, tag="w1", bufs=6, name="w1t")
                        nc.sync.dma_start(w1l_p, w1_ap(w1l_d, ne, 0))
                        prefetched[(ntg, ne)] = (w1h_p, w1l_p)
                    emit_mm2(tg, e, hgh, hgl, yac)
                for cb in range(CB):
                    for ti in range(TI):
                        nc.gpsimd.dma_start(
                            outT_d[
                                cb * 128 : (cb + 1) * 128,
                                tg * TG + ti * 512 : tg * TG + (ti + 1) * 512,
                            ],
                            yac[:, cb, ti * 512 : (ti + 1) * 512],
                        )

    nc.compile()
    return nc


def _get_nc():
    global _CACHED_NC
    if _CACHED_NC is None:
        _CACHED_NC = build_nc()
    return _CACHED_NC


def _split8(a, scale):
    """Dekker 2-term fp8 split of scale*a. Returns (hi, lo) as e4m3."""
    s = (a * scale).astype(np.float32)
    hi = s.astype(NP8)
    lo = (s - hi.astype(np.float32)).astype(NP8)
    return hi, lo


def make_in_maps(x, gate_w, gate_b, w1, b1, w2, b2):
    x = np.asarray(x, np.float32)
    gate_w = np.asarray(gate_w, np.float32)
    gate_b = np.asarray(gate_b, np.float32)
    w1 = np.asarray(w1, np.float32)
    b1 = np.asarray(b1, np.float32)
    w2 = np.asarray(w2, np.float32)
    b2 = np.asarray(b2, np.float32)

    # pack x as [tg, p, cc*TG]: contiguous 1KB+ runs per partition
    xT = x.reshape(N, C).T.reshape(CB, 128, TCG, TG).transpose(2, 1, 0, 3)
    xT = np.ascontiguousarray(xT).reshape(TCG, 128, CB * TG)
    xh, xl = _split8(xT, 1.0)

    ones32 = np.full((E, EL), S2, np.float16)

    in_maps = []
    for i in range(NCORES):
        lo, hi = EL * i, EL * (i + 1)
        perm = list(range(lo, hi)) + [e for e in range(E) if not (lo <= e < hi)]
        gwh, gwl = _split8(np.ascontiguousarray(gate_w[:, perm]), S1)
        # w1 packed [e, hbg, p, cc*512]; w2 packed [e, cb, p, hb*128]
        w1p = w1[lo:hi].reshape(EL, CB, 128, HB // 4, 512).transpose(0, 3, 2, 1, 4)
        w1p = np.ascontiguousarray(w1p).reshape(EL, HB // 4, 128, CB * 512)
        w2p = w2[lo:hi].reshape(EL, HB, 128, CB, 128).transpose(0, 3, 2, 1, 4)
        w2p = np.ascontiguousarray(w2p).reshape(EL, CB, 128, HB * 128)
        w1h, w1l = _split8(w1p, S1)
        w2h, w2l = _split8(w2p, S2)
        in_maps.append(
            {
                "xh": xh,
                "xl": xl,
                "gwh": gwh,
                "gwl": gwl,
                "gb": np.ascontiguousarray(gate_b[perm]).reshape(E, 1),
                "ones32": ones32,
                "w1h": w1h,
                "w1l": w1l,
                "b1": np.ascontiguousarray(
                    b1[lo:hi].reshape(EL, HB, 128).transpose(2, 0, 1)
                ),
                "w2h": w2h,
                "w2l": w2l,
                "b2P": np.ascontiguousarray(
                    (S2 * b2[lo:hi]).reshape(EL, CB, 128).transpose(2, 0, 1)
                ),
            }
        )
    return in_maps


def kernel(x, gate_w, gate_b, w1, b1, w2, b2, _trace=False, _tmpdir=None):
    nc = _get_nc()
    in_maps = make_in_maps(x, gate_w, gate_b, w1, b1, w2, b2)
    res = run_bass_kernel_spmd(
        nc,
        in_maps,
        core_ids=list(range(NCORES)),
        trace=_trace,
        tmpdir=_tmpdir,
    )
    acc = res.results[0]["outT"].astype(np.float64)
    for r in res.results[1:]:
        acc += r["outT"]
    out = acc.T.reshape(B, T, C).astype(np.float32)
    if _trace:
        kernel._last_results = res
    return out
